# revision 24
# baseline (speedup 1.0000x reference)
"""AttentionBlock (GroupNorm + single-head full attention + residual) on 8 TRN2 cores.

Data-parallel: batch B=8, one sample per NeuronCore. Per core:
  x [256, 4096] f32 -> groupnorm -> h (fp8 e4m3)
  Algebraic folding (host-precomputed weight products):
    S[q,k] = q.k = sum_c h[c,q]*G2[c,k] + w[k] + c0
       G2 = M h + v,  M = Wq^T Wk, v = Wq^T b_k,  w[k] = (Wk^T b_q).h_k, c0 = b_q.b_k
    out_pre[q,co] = sum_k P[k,q]*VV[co,k],  VV = (Wo Wv) h + Wo b_v   (proj_out folded)
  All heavy matmuls run in fp8 e4m3 with MatmulPerfMode.DoubleRow (contraction
  over 2 k-subtiles per instruction, 2x PE throughput).  The per-k score bias
  w[k]+c0 is folded multiplicatively into VV (f[k] = exp(SCALE*(w[k]+c0)),
  sum_k e*f*vv == sum_k (e*f)*vv), which makes the softmax-exp bias a constant
  (-SHIFT) so each ACT exp instruction can span two PSUM banks (1024 wide).
  The softmax denominator rides as a ones-column of VV (scaled by f).  P^T
  layout [k, q] comes straight out of the S^T matmul so the 4096x4096 attention
  matrix is never transposed; only the final [4096, 256] attention output is
  transposed back to [c, n] via TensorE.
"""

import numpy as np
import ml_dtypes

import concourse.bacc as bacc
import concourse.bass as bass
import concourse.tile as tile
from concourse import mybir
from concourse.bass_utils import run_bass_kernel_spmd

F32 = mybir.dt.float32
BF16 = mybir.dt.bfloat16
F8 = mybir.dt.float8e4
AF = mybir.ActivationFunctionType
DR = mybir.MatmulPerfMode.DoubleRow
ALU = mybir.AluOpType
F8NP = ml_dtypes.float8_e4m3fn

C = 256          # channels
N = 4096         # spatial (64*64)
P = 128          # partitions
CT = C // P      # channel tiles (2)
NG = 8           # groups
GS = C // NG     # group size (32)
EPS = 1e-5
QB = 512         # queries per block
NQB = N // QB    # 8
NKT = N // P     # 32 k-tiles
NPR = NKT // 2   # 16 k-tile pairs
SCALE = 1.0 / np.sqrt(C)  # 1/16
SHIFT = 3.0      # global exp shift (softmax-invariant), keeps fp8 e in range


def _group_masks():
    g0 = np.zeros((P, NG), np.float32)
    g1 = np.zeros((P, NG), np.float32)
    for p in range(P):
        g0[p, p // GS] = 1.0
        g1[p, 4 + p // GS] = 1.0
    return g0, g1


def build_nc():
    nc = bacc.Bacc("TRN2", target_bir_lowering=False)

    x_d = nc.dram_tensor("x", [C, N], F32, kind="ExternalInput")
    wcat_d = nc.dram_tensor("wcat", [P, CT, 514], F8, kind="ExternalInput")
    fcat_d = nc.dram_tensor("fcat", [P, 146], F32, kind="ExternalInput")
    gcat_d = nc.dram_tensor("gcat", [NG, 2 * P], F32, kind="ExternalInput")
    out_d = nc.dram_tensor("out", [C, N], F32, kind="ExternalOutput")

    import contextlib
    with tile.TileContext(nc) as tc, contextlib.ExitStack() as ctx:
        cst = ctx.enter_context(tc.tile_pool(name="cst", bufs=1))
        big = ctx.enter_context(tc.tile_pool(name="big", bufs=1))
        e4p = ctx.enter_context(tc.tile_pool(name="e4p", bufs=3))
        anp = ctx.enter_context(tc.tile_pool(name="anp", bufs=4))
        outp = ctx.enter_context(tc.tile_pool(name="outp", bufs=2))
        sml = ctx.enter_context(tc.tile_pool(name="sml", bufs=2))
        ps_s = ctx.enter_context(tc.tile_pool(name="ps_s", bufs=2, space="PSUM"))
        ps_o = ctx.enter_context(tc.tile_pool(name="ps_o", bufs=3, space="PSUM"))
        ps_t = ctx.enter_context(tc.tile_pool(name="ps_t", bufs=1, space="PSUM"))

        # ---- x load FIRST (descriptor generation on the SP queue is serial
        # ~500ns/DMA, so x transfers must start before the const loads).
        # 4 half-tiles (ct x half) in 1024-col chunks so the groupnorm stats
        # start as soon as each tile's DMAs land ----
        NH = N // 2
        x_t = {(ct, h): big.tile([P, NH], F32, name=f"x_{ct}_{h}")
               for ct in range(CT) for h in range(2)}
        x_r = x_d.rearrange("(t p) n -> p t n", p=P)
        for h in range(2):
            for xc in range(2):
                for ct in range(CT):
                    lo = xc * 1024
                    nc.sync.dma_start(
                        out=x_t[(ct, h)][:, lo:lo + 1024],
                        in_=x_r[:, ct, h * NH + lo:h * NH + lo + 1024],
                    )

        def x_slice(ct, n0, w):
            h, lo = divmod(n0, NH)
            return x_t[(ct, h)][:, lo:lo + w]

        # ---- const loads (3 packed DMAs) ----
        mt8_sb = cst.tile([P, CT, C], F8, name="mt8_sb")
        nc.sync.dma_start(out=mt8_sb, in_=wcat_d[:, :, 0:256])
        w2t8_sb = cst.tile([P, CT, 257], F8, name="w2t8_sb")
        nc.sync.dma_start(out=w2t8_sb, in_=wcat_d[:, :, 256:513])
        fcat_sb = cst.tile([P, 146], F32, name="fcat_sb")
        nc.sync.dma_start(out=fcat_sb, in_=fcat_d[:, :])
        bo_sb = fcat_sb[:, 0:2]
        g0_sb = fcat_sb[:, 2:10]
        g1_sb = fcat_sb[:, 10:18]
        eye_sb = fcat_sb[:, 18:146]
        gcat_sb = cst.tile([NG, 2 * P], F32, name="gcat_sb")
        nc.sync.dma_start(out=gcat_sb, in_=gcat_d[:, :])
        gt0_sb = gcat_sb[:, 0:P]
        gt1_sb = gcat_sb[:, P:2 * P]
        eyeb = cst.tile([P, P], BF16, name="eyeb")
        nc.vector.tensor_copy(out=eyeb, in_=eye_sb)

        eps_sb = cst.tile([NG, 1], F32, name="eps_sb")
        nc.vector.memset(eps_sb, EPS)
        warm = cst.tile([NG, 1], F32, name="warm")
        nc.scalar.activation(out=warm, in_=eps_sb, func=AF.Sqrt, bias=eps_sb)
        nshift = cst.tile([P, 1], F32, name="nshift")
        nc.vector.memset(nshift, -SHIFT)

        # ---- groupnorm stats ----
        NSG = N // 512
        stats = sml.tile([P, CT, NSG, 6], F32, name="stats")
        mv = sml.tile([P, CT, 2], F32, name="mv")
        for h in range(2):
            for sg4 in range(NSG // 2):
                for ct in range(CT):
                    sg = h * (NSG // 2) + sg4
                    nc.vector.bn_stats(
                        out=stats[:, ct, sg, :],
                        in_=x_slice(ct, sg * 512, 512),
                    )
        for ct in range(CT):
            nc.vector.bn_aggr(out=mv[:, ct, :], in_=stats[:, ct, :, :])
        st3 = sml.tile([P, CT, 3], F32, name="st3")
        for ct in range(CT):
            nc.vector.tensor_copy(out=st3[:, ct, 0:2], in_=mv[:, ct, :])
            nc.vector.tensor_mul(
                out=st3[:, ct, 2:3], in0=mv[:, ct, 0:1], in1=mv[:, ct, 0:1]
            )
        gps = ps_s.tile([NG, 3], F32, name="gps", tag="s")
        nc.tensor.matmul(gps, lhsT=g0_sb, rhs=st3[:, 0, :], start=True, stop=False)
        nc.tensor.matmul(gps, lhsT=g1_sb, rhs=st3[:, 1, :], start=False, stop=True)
        gsb = sml.tile([NG, 3], F32, name="gsb")
        nc.vector.tensor_copy(out=gsb, in_=gps)
        gmean = sml.tile([NG, 1], F32, name="gmean")
        nc.vector.tensor_scalar_mul(out=gmean, in0=gsb[:, 0:1], scalar1=1.0 / GS)
        gtmp = sml.tile([NG, 1], F32, name="gtmp")
        nc.vector.tensor_add(out=gtmp, in0=gsb[:, 1:2], in1=gsb[:, 2:3])
        nc.vector.tensor_scalar_mul(out=gtmp, in0=gtmp, scalar1=1.0 / GS)
        gmsq = sml.tile([NG, 1], F32, name="gmsq")
        nc.vector.tensor_mul(out=gmsq, in0=gmean, in1=gmean)
        gvar = sml.tile([NG, 1], F32, name="gvar")
        nc.vector.tensor_sub(out=gvar, in0=gtmp, in1=gmsq)
        gstd = sml.tile([NG, 1], F32, name="gstd")
        nc.scalar.activation(out=gstd, in_=gvar, func=AF.Sqrt, bias=eps_sb)
        grstd = sml.tile([NG, 1], F32, name="grstd")
        nc.vector.reciprocal(out=grstd, in_=gstd)
        gpar = sml.tile([NG, 2], F32, name="gpar")
        nc.vector.tensor_copy(out=gpar[:, 0:1], in_=gmean)
        nc.vector.tensor_copy(out=gpar[:, 1:2], in_=grstd)
        mr_sb = sml.tile([P, CT, 2], F32, name="mr_sb")
        for ct, gt in ((0, gt0_sb), (1, gt1_sb)):
            bps = ps_s.tile([P, 2], F32, name=f"bps{ct}", tag="s")
            nc.tensor.matmul(bps, lhsT=gt, rhs=gpar, start=True, stop=True)
            nc.vector.tensor_copy(out=mr_sb[:, ct, :], in_=bps)
        # h = (x - mean) * rstd -> fp8, pipelined per 512-col block with G2
        hf8 = big.tile([P, CT, N], F8, name="hf8")
        g2f8 = big.tile([P, CT, N], F8, name="g2f8")
        for kb in range(NQB):
            ks = slice(kb * QB, (kb + 1) * QB)
            for ct in range(CT):
                nc.vector.tensor_scalar(
                    out=hf8[:, ct, ks],
                    in0=x_slice(ct, kb * QB, QB),
                    scalar1=mr_sb[:, ct, 0:1],
                    scalar2=mr_sb[:, ct, 1:2],
                    op0=ALU.subtract,
                    op1=ALU.mult,
                )
            g2ps = ps_s.tile([P, CT, QB], F32, name=f"g2ps_{kb}", tag="s")
            for ct in range(CT):
                nc.tensor.matmul(
                    g2ps[:, ct, :], lhsT=mt8_sb[:, :, ct * P:(ct + 1) * P],
                    rhs=hf8[:, :, ks], start=True, stop=True, perf_mode=DR,
                )
            nc.scalar.mul(g2f8[:, :, ks], g2ps, 1.0 / 16.0)

        # ---- VV projection + f = exp(SCALE*(w+c0)) folding ----
        # vv8[k, 0:256] = 16*VV[k, co]*f[k]; vv8[k, 256] = f[k]
        vv8 = big.tile([P, NKT, 257], F8, name="vv8")
        fz = big.tile([P, NKT], F32, name="fz")
        ftmp = big.tile([P, NKT], F32, name="ftmp")
        fex = big.tile([P, NKT], F32, name="fex")

        def emit_vv_mm(kt):
            vps = ps_o.tile([P, 257], F32, name=f"vps_{kt}", tag="o")
            ks = slice(kt * P, (kt + 1) * P)
            nc.tensor.matmul(vps, lhsT=hf8[:, :, ks], rhs=w2t8_sb,
                             start=True, stop=True, perf_mode=DR)
            return vps

        def emit_vv_pair(i):
            kt0, kt1 = 2 * i, 2 * i + 1
            vps0 = emit_vv_mm(kt0)
            vps1 = emit_vv_mm(kt1)
            for kt, vps in ((kt0, vps0), (kt1, vps1)):
                nc.vector.tensor_scalar_mul(
                    out=fz[:, kt:kt + 1], in0=vps[:, 256:257],
                    scalar1=float(SCALE / 16.0),
                )
            pr = slice(kt0, kt1 + 1)
            nc.vector.tensor_scalar(
                out=ftmp[:, pr], in0=fz[:, pr], scalar1=0.5, scalar2=1.0,
                op0=ALU.mult, op1=ALU.add,
            )
            nc.vector.tensor_mul(out=fex[:, pr], in0=ftmp[:, pr], in1=fz[:, pr])
            nc.vector.tensor_scalar_add(out=fex[:, pr], in0=fex[:, pr], scalar1=1.0)
            for kt, vps in ((kt0, vps0), (kt1, vps1)):
                nc.vector.tensor_scalar_mul(
                    out=vv8[:, kt, 0:256], in0=vps[:, 0:256],
                    scalar1=fex[:, kt:kt + 1],
                )
                nc.vector.tensor_scalar_mul(
                    out=vv8[:, kt, 256:257], in0=fex[:, kt:kt + 1], scalar1=1.0
                )

        # ---- attention ----
        e4_tiles = {}
        o_cur = {}

        def emit_s_pair(qb, t):
            qs_ = slice(qb * QB, (qb + 1) * QB)
            sp = ps_s.tile([P, 2, QB], F32, name=f"sps_{qb}_{t}", tag="s")
            for j in (0, 1):
                kt = 2 * t + j
                nc.tensor.matmul(
                    sp[:, j, :], lhsT=g2f8[:, :, kt * P:(kt + 1) * P],
                    rhs=hf8[:, :, qs_], start=True, stop=True, perf_mode=DR,
                )
            nc.scalar.activation(
                out=e4_tiles[qb][:, 2 * t:2 * t + 2, :], in_=sp,
                func=AF.Exp, scale=float(SCALE), bias=nshift,
            )

        def emit_pv(qb, qs, t):
            if t == 0:
                o_cur[qs] = ps_o.tile([P, 257], F32, name=f"ops_{qb}_{qs}", tag="o")
            nc.tensor.matmul(
                o_cur[qs],
                lhsT=e4_tiles[qb][:, 2 * t:2 * t + 2, qs * P:(qs + 1) * P],
                rhs=vv8[:, 2 * t:2 * t + 2, :],
                start=(t == 0), stop=(t == NPR - 1), perf_mode=DR,
            )

        def emit_ep_a(qb, qs):
            o = o_cur[qs]
            recip = sml.tile([P, 1], F32, name=f"rc_{qb}_{qs}", tag="recip")
            nc.vector.reciprocal(out=recip, in_=o[:, 256:257])
            attn = anp.tile([P, C], BF16, name=f"attn_{qb}_{qs}", tag="attn")
            nc.vector.tensor_scalar_mul(out=attn, in0=o[:, 0:256], scalar1=recip)
            return attn

        def emit_ep_b(qb, qs, attn):
            if qs == 0:
                tps_cur[qb] = ps_t.tile(
                    [P, CT, QB], BF16, name=f"tps_{qb}", tag="t"
                )
            tps = tps_cur[qb]
            for ct in range(CT):
                nc.tensor.transpose(
                    tps[:, ct, qs * P:(qs + 1) * P],
                    attn[:, ct * P:(ct + 1) * P],
                    eyeb,
                )
            if qs == 3:
                emit_qb_out(qb, tps_cur.pop(qb))

        def emit_qb_out(qb, tps):
            outt = outp.tile([P, CT, QB], F32, name=f"outt_{qb}", tag="outt")
            qs_ = slice(qb * QB, (qb + 1) * QB)
            for ct in range(CT):
                nc.vector.tensor_scalar(
                    out=outt[:, ct, :], in0=tps[:, ct, :],
                    scalar1=1.0 / 16.0, scalar2=bo_sb[:, ct:ct + 1],
                    op0=ALU.mult, op1=ALU.add,
                )
                nc.vector.tensor_add(
                    out=outt[:, ct, :], in0=outt[:, ct, :],
                    in1=x_slice(ct, qb * QB, QB),
                )
            out_r = out_d.rearrange("(t p) n -> p t n", p=P)
            nc.gpsimd.dma_start(out=out_r[:, :, qs_], in_=outt)

        # aux work interleaved into the S phase of each q-block:
        #   qb 0: the 32 VV matmul groups; qb >= 1: the 64 PV matmuls of qb-1.
        tps_cur = {}
        pending_b = []

        def aux_pv(qb_prev, i):  # i in 0..15 -> 4 PV matmuls per step
            new_b = []
            for k in range(4):
                idx = 4 * i + k
                qs, t = divmod(idx, NPR)
                emit_pv(qb_prev, qs, t)
                if t == NPR - 1:
                    attn = emit_ep_a(qb_prev, qs)
                    new_b.append((qb_prev, qs, attn))
            while pending_b:
                emit_ep_b(*pending_b.pop(0))
            pending_b.extend(new_b)

        for qb in range(NQB):
            e4_tiles[qb] = e4p.tile([P, NKT, QB], F8, name=f"e4_{qb}", tag="e4")
            if qb >= 3:
                del e4_tiles[qb - 3]
            for t in range(NPR):
                emit_s_pair(qb, t)
                if qb == 0:
                    emit_vv_pair(t)
                else:
                    aux_pv(qb - 1, t)
        for i in range(NPR):
            aux_pv(NQB - 1, i)
        while pending_b:
            emit_ep_b(*pending_b.pop(0))

    nc.compile()
    return nc


_NC = None


def _get_nc():
    global _NC
    if _NC is None:
        _NC = build_nc()
    return _NC


def _host_prep(x, w_q, b_q, w_k, b_k, w_v, b_v, w_o, b_o):
    x = np.ascontiguousarray(np.asarray(x, np.float32))
    B = x.shape[0]
    wq = np.asarray(w_q, np.float32)
    wk = np.asarray(w_k, np.float32)
    wv = np.asarray(w_v, np.float32)
    wo = np.asarray(w_o, np.float32)
    bq = np.asarray(b_q, np.float32)
    bk = np.asarray(b_k, np.float32)
    bv = np.asarray(b_v, np.float32)
    bo = np.asarray(b_o, np.float32)

    def to_pt(a):  # [C, ...] -> [P, CT, ...]
        return np.ascontiguousarray(
            a.reshape(CT, P, *a.shape[1:]).transpose(1, 0, *range(2, a.ndim + 1))
        )

    mt = (wk.T @ wq).astype(np.float32)       # lhsT[c, c'] = M[c', c]
    mt8 = to_pt((16.0 * mt).astype(F8NP))
    u = (wk.T @ bq).astype(np.float32)
    c0 = float(bq @ bk)
    w2 = (wo @ wv).astype(np.float32)
    b2 = (wo @ bv).astype(np.float32)
    w2t = np.zeros((C, 257), np.float32)
    w2t[:, :256] = 16.0 * w2.T
    w2t[:, 256] = 16.0 * u
    w2t8 = to_pt(w2t.astype(F8NP))
    bo = bo + b2   # sum_k softmax = 1 -> Wo b_v folds into the output bias
    pad = np.zeros((P, CT, 1), F8NP)
    wcat = np.concatenate([mt8, w2t8, pad], axis=2)     # [P, CT, 514] f8 (even stride)
    g0_np, g1_np = _group_masks()
    fcat = np.zeros((P, 146), np.float32)
    fcat[:, 0:2] = to_pt(bo)
    fcat[:, 2:10] = g0_np
    fcat[:, 10:18] = g1_np
    fcat[:, 18:146] = np.eye(P, dtype=np.float32)
    gcat = np.concatenate(
        [np.ascontiguousarray(g0_np.T), np.ascontiguousarray(g1_np.T)], axis=1
    )

    xr = x.reshape(B, C, N)
    shared = {
        "wcat": np.ascontiguousarray(wcat),
        "fcat": fcat,
        "gcat": np.ascontiguousarray(gcat.astype(np.float32)),
    }
    in_maps = [{"x": np.ascontiguousarray(xr[i]), **shared} for i in range(B)]
    return x, in_maps


def kernel(x, w_q, b_q, w_k, b_k, w_v, b_v, w_o, b_o):
    x, in_maps = _host_prep(x, w_q, b_q, w_k, b_k, w_v, b_v, w_o, b_o)
    B = x.shape[0]
    nc = _get_nc()
    res = run_bass_kernel_spmd(nc, in_maps, core_ids=list(range(B)))
    global _LAST
    _LAST = res
    out = np.stack([res.results[i]["out"] for i in range(B)], axis=0)
    return out.reshape(x.shape).astype(np.float32)


_LAST = None


# revision 25
# speedup vs baseline: 1.1579x; 1.1579x over previous
"""AttentionBlock (GroupNorm + single-head full attention + residual) on 8 TRN2 cores.

Data-parallel: batch B=8, one sample per NeuronCore. Per core:
  x [256, 4096] f32 -> groupnorm -> h (fp8 e4m3)
  Algebraic folding (host-precomputed weight products):
    S[q,k] = q.k = sum_c h[c,q]*G2[c,k] + w[k] + c0
       G2 = M h + v,  M = Wq^T Wk, v = Wq^T b_k,  w[k] = (Wk^T b_q).h_k, c0 = b_q.b_k
    out_pre[q,co] = sum_k P[k,q]*VV[co,k],  VV = (Wo Wv) h + Wo b_v   (proj_out folded)
  All heavy matmuls run in fp8 e4m3 with MatmulPerfMode.DoubleRow (contraction
  over 2 k-subtiles per instruction, 2x PE throughput).  The per-k score bias
  w[k]+c0 is folded multiplicatively into VV (f[k] = exp(SCALE*(w[k]+c0)),
  sum_k e*f*vv == sum_k (e*f)*vv), which makes the softmax-exp bias a constant
  (-SHIFT) so each ACT exp instruction can span two PSUM banks (1024 wide).
  The softmax denominator rides as a ones-column of VV (scaled by f).  P^T
  layout [k, q] comes straight out of the S^T matmul so the 4096x4096 attention
  matrix is never transposed; only the final [4096, 256] attention output is
  transposed back to [c, n] via TensorE.
"""

import numpy as np
import ml_dtypes

import concourse.bacc as bacc
import concourse.bass as bass
import concourse.tile as tile
from concourse import mybir
from concourse.bass_utils import run_bass_kernel_spmd

F32 = mybir.dt.float32
BF16 = mybir.dt.bfloat16
F8 = mybir.dt.float8e4
AF = mybir.ActivationFunctionType
DR = mybir.MatmulPerfMode.DoubleRow
ALU = mybir.AluOpType
F8NP = ml_dtypes.float8_e4m3fn

C = 256          # channels
N = 4096         # spatial (64*64)
P = 128          # partitions
CT = C // P      # channel tiles (2)
NG = 8           # groups
GS = C // NG     # group size (32)
EPS = 1e-5
QB = 512         # queries per block
NQB = N // QB    # 8
NKT = N // P     # 32 k-tiles
NPR = NKT // 2   # 16 k-tile pairs
SCALE = 1.0 / np.sqrt(C)  # 1/16
SHIFT = 3.0      # global exp shift (softmax-invariant), keeps fp8 e in range


def _group_masks():
    g0 = np.zeros((P, NG), np.float32)
    g1 = np.zeros((P, NG), np.float32)
    for p in range(P):
        g0[p, p // GS] = 1.0
        g1[p, 4 + p // GS] = 1.0
    return g0, g1


def build_nc():
    nc = bacc.Bacc("TRN2", target_bir_lowering=False)

    x_d = nc.dram_tensor("x", [C, N], F32, kind="ExternalInput")
    mt8_d = nc.dram_tensor("mt8", [P, CT, C], F8, kind="ExternalInput")
    w2t8_d = nc.dram_tensor("w2t8", [P, CT, 257], F8, kind="ExternalInput")
    bo_d = nc.dram_tensor("bo", [P, CT], F32, kind="ExternalInput")
    out_d = nc.dram_tensor("out", [C, N], F32, kind="ExternalOutput")

    g0_np, g1_np = _group_masks()
    g0_d = nc.inline_tensor(g0_np, name="g0c")
    g1_d = nc.inline_tensor(g1_np, name="g1c")
    gt0_d = nc.inline_tensor(np.ascontiguousarray(g0_np.T), name="gt0c")
    gt1_d = nc.inline_tensor(np.ascontiguousarray(g1_np.T), name="gt1c")
    eye_d = nc.inline_tensor(np.eye(P, dtype=np.float32), name="eyec")

    import contextlib
    with tile.TileContext(nc) as tc, contextlib.ExitStack() as ctx:
        cst = ctx.enter_context(tc.tile_pool(name="cst", bufs=1))
        big = ctx.enter_context(tc.tile_pool(name="big", bufs=1))
        e4p = ctx.enter_context(tc.tile_pool(name="e4p", bufs=3))
        anp = ctx.enter_context(tc.tile_pool(name="anp", bufs=4))
        outp = ctx.enter_context(tc.tile_pool(name="outp", bufs=2))
        sml = ctx.enter_context(tc.tile_pool(name="sml", bufs=2))
        ps_s = ctx.enter_context(tc.tile_pool(name="ps_s", bufs=2, space="PSUM"))
        ps_o = ctx.enter_context(tc.tile_pool(name="ps_o", bufs=3, space="PSUM"))
        ps_t = ctx.enter_context(tc.tile_pool(name="ps_t", bufs=1, space="PSUM"))

        # ---- const loads ----
        mt8_sb = cst.tile([P, CT, C], F8, name="mt8_sb")
        nc.sync.dma_start(out=mt8_sb, in_=mt8_d[:, :, :])
        w2t8_sb = cst.tile([P, CT, 257], F8, name="w2t8_sb")
        nc.sync.dma_start(out=w2t8_sb, in_=w2t8_d[:, :, :])
        bo_sb = cst.tile([P, CT], F32, name="bo_sb")
        nc.sync.dma_start(out=bo_sb, in_=bo_d[:, :])

        eye_sb = cst.tile([P, P], F32, name="eye_sb")
        nc.sync.dma_start(out=eye_sb, in_=eye_d[:, :])
        eyeb = cst.tile([P, P], BF16, name="eyeb")
        nc.vector.tensor_copy(out=eyeb, in_=eye_sb)

        g0_sb = cst.tile([P, NG], F32, name="g0_sb")
        nc.sync.dma_start(out=g0_sb, in_=g0_d[:, :])
        g1_sb = cst.tile([P, NG], F32, name="g1_sb")
        nc.sync.dma_start(out=g1_sb, in_=g1_d[:, :])
        gt0_sb = cst.tile([NG, P], F32, name="gt0_sb")
        nc.sync.dma_start(out=gt0_sb, in_=gt0_d[:, :])
        gt1_sb = cst.tile([NG, P], F32, name="gt1_sb")
        nc.sync.dma_start(out=gt1_sb, in_=gt1_d[:, :])

        eps_sb = cst.tile([NG, 1], F32, name="eps_sb")
        nc.vector.memset(eps_sb, EPS)
        warm = cst.tile([NG, 1], F32, name="warm")
        nc.scalar.activation(out=warm, in_=eps_sb, func=AF.Sqrt, bias=eps_sb)
        nshift = cst.tile([P, 1], F32, name="nshift")
        nc.vector.memset(nshift, -SHIFT)

        # ---- x load: 4 half-tiles (ct x half) in 512-col chunks so the
        # groupnorm stats can start as soon as each tile's DMAs land ----
        NH = N // 2
        x_t = {(ct, h): big.tile([P, NH], F32, name=f"x_{ct}_{h}")
               for ct in range(CT) for h in range(2)}
        x_r = x_d.rearrange("(t p) n -> p t n", p=P)
        for h in range(2):
            for xc in range(4):
                for ct in range(CT):
                    lo = xc * 512
                    nc.sync.dma_start(
                        out=x_t[(ct, h)][:, lo:lo + 512],
                        in_=x_r[:, ct, h * NH + lo:h * NH + lo + 512],
                    )

        def x_slice(ct, n0, w):
            h, lo = divmod(n0, NH)
            return x_t[(ct, h)][:, lo:lo + w]

        # ---- groupnorm stats ----
        NSG = N // 512
        stats = sml.tile([P, CT, NSG, 6], F32, name="stats")
        mv = sml.tile([P, CT, 2], F32, name="mv")
        for h in range(2):
            for sg4 in range(NSG // 2):
                for ct in range(CT):
                    sg = h * (NSG // 2) + sg4
                    nc.vector.bn_stats(
                        out=stats[:, ct, sg, :],
                        in_=x_slice(ct, sg * 512, 512),
                    )
        for ct in range(CT):
            nc.vector.bn_aggr(out=mv[:, ct, :], in_=stats[:, ct, :, :])
        st3 = sml.tile([P, CT, 3], F32, name="st3")
        for ct in range(CT):
            nc.vector.tensor_copy(out=st3[:, ct, 0:2], in_=mv[:, ct, :])
            nc.vector.tensor_mul(
                out=st3[:, ct, 2:3], in0=mv[:, ct, 0:1], in1=mv[:, ct, 0:1]
            )
        gps = ps_s.tile([NG, 3], F32, name="gps", tag="s")
        nc.tensor.matmul(gps, lhsT=g0_sb, rhs=st3[:, 0, :], start=True, stop=False)
        nc.tensor.matmul(gps, lhsT=g1_sb, rhs=st3[:, 1, :], start=False, stop=True)
        gsb = sml.tile([NG, 3], F32, name="gsb")
        nc.vector.tensor_copy(out=gsb, in_=gps)
        gmean = sml.tile([NG, 1], F32, name="gmean")
        nc.vector.tensor_scalar_mul(out=gmean, in0=gsb[:, 0:1], scalar1=1.0 / GS)
        gtmp = sml.tile([NG, 1], F32, name="gtmp")
        nc.vector.tensor_add(out=gtmp, in0=gsb[:, 1:2], in1=gsb[:, 2:3])
        nc.vector.tensor_scalar_mul(out=gtmp, in0=gtmp, scalar1=1.0 / GS)
        gmsq = sml.tile([NG, 1], F32, name="gmsq")
        nc.vector.tensor_mul(out=gmsq, in0=gmean, in1=gmean)
        gvar = sml.tile([NG, 1], F32, name="gvar")
        nc.vector.tensor_sub(out=gvar, in0=gtmp, in1=gmsq)
        gstd = sml.tile([NG, 1], F32, name="gstd")
        nc.scalar.activation(out=gstd, in_=gvar, func=AF.Sqrt, bias=eps_sb)
        grstd = sml.tile([NG, 1], F32, name="grstd")
        nc.vector.reciprocal(out=grstd, in_=gstd)
        gpar = sml.tile([NG, 2], F32, name="gpar")
        nc.vector.tensor_copy(out=gpar[:, 0:1], in_=gmean)
        nc.vector.tensor_copy(out=gpar[:, 1:2], in_=grstd)
        mr_sb = sml.tile([P, CT, 2], F32, name="mr_sb")
        for ct, gt in ((0, gt0_sb), (1, gt1_sb)):
            bps = ps_s.tile([P, 2], F32, name=f"bps{ct}", tag="s")
            nc.tensor.matmul(bps, lhsT=gt, rhs=gpar, start=True, stop=True)
            nc.vector.tensor_copy(out=mr_sb[:, ct, :], in_=bps)
        # h = (x - mean) * rstd -> fp8, pipelined per 512-col block with G2
        hf8 = big.tile([P, CT, N], F8, name="hf8")
        g2f8 = big.tile([P, CT, N], F8, name="g2f8")
        for kb in range(NQB):
            ks = slice(kb * QB, (kb + 1) * QB)
            for ct in range(CT):
                nc.vector.tensor_scalar(
                    out=hf8[:, ct, ks],
                    in0=x_slice(ct, kb * QB, QB),
                    scalar1=mr_sb[:, ct, 0:1],
                    scalar2=mr_sb[:, ct, 1:2],
                    op0=ALU.subtract,
                    op1=ALU.mult,
                )
            g2ps = ps_s.tile([P, CT, QB], F32, name=f"g2ps_{kb}", tag="s")
            for ct in range(CT):
                nc.tensor.matmul(
                    g2ps[:, ct, :], lhsT=mt8_sb[:, :, ct * P:(ct + 1) * P],
                    rhs=hf8[:, :, ks], start=True, stop=True, perf_mode=DR,
                )
            nc.scalar.mul(g2f8[:, :, ks], g2ps, 1.0 / 16.0)

        # ---- VV projection + f = exp(SCALE*(w+c0)) folding ----
        # vv8[k, 0:256] = 16*VV[k, co]*f[k]; vv8[k, 256] = f[k]
        vv8 = big.tile([P, NKT, 257], F8, name="vv8")
        fz = big.tile([P, NKT], F32, name="fz")
        ftmp = big.tile([P, NKT], F32, name="ftmp")
        fex = big.tile([P, NKT], F32, name="fex")

        def emit_vv_mm(kt):
            vps = ps_o.tile([P, 257], F32, name=f"vps_{kt}", tag="o")
            ks = slice(kt * P, (kt + 1) * P)
            nc.tensor.matmul(vps, lhsT=hf8[:, :, ks], rhs=w2t8_sb,
                             start=True, stop=True, perf_mode=DR)
            return vps

        def emit_vv_pair(i):
            kt0, kt1 = 2 * i, 2 * i + 1
            vps0 = emit_vv_mm(kt0)
            vps1 = emit_vv_mm(kt1)
            for kt, vps in ((kt0, vps0), (kt1, vps1)):
                nc.vector.tensor_scalar_mul(
                    out=fz[:, kt:kt + 1], in0=vps[:, 256:257],
                    scalar1=float(SCALE / 16.0),
                )
            pr = slice(kt0, kt1 + 1)
            nc.vector.tensor_scalar(
                out=ftmp[:, pr], in0=fz[:, pr], scalar1=0.5, scalar2=1.0,
                op0=ALU.mult, op1=ALU.add,
            )
            nc.vector.tensor_mul(out=fex[:, pr], in0=ftmp[:, pr], in1=fz[:, pr])
            nc.vector.tensor_scalar_add(out=fex[:, pr], in0=fex[:, pr], scalar1=1.0)
            for kt, vps in ((kt0, vps0), (kt1, vps1)):
                nc.vector.tensor_scalar_mul(
                    out=vv8[:, kt, 0:256], in0=vps[:, 0:256],
                    scalar1=fex[:, kt:kt + 1],
                )
                nc.vector.tensor_scalar_mul(
                    out=vv8[:, kt, 256:257], in0=fex[:, kt:kt + 1], scalar1=1.0
                )

        # ---- attention ----
        e4_tiles = {}
        o_cur = {}

        def emit_s_pair(qb, t):
            qs_ = slice(qb * QB, (qb + 1) * QB)
            sp = ps_s.tile([P, 2, QB], F32, name=f"sps_{qb}_{t}", tag="s")
            for j in (0, 1):
                kt = 2 * t + j
                nc.tensor.matmul(
                    sp[:, j, :], lhsT=g2f8[:, :, kt * P:(kt + 1) * P],
                    rhs=hf8[:, :, qs_], start=True, stop=True, perf_mode=DR,
                )
            nc.scalar.activation(
                out=e4_tiles[qb][:, 2 * t:2 * t + 2, :], in_=sp,
                func=AF.Exp, scale=float(SCALE), bias=nshift,
            )

        def emit_pv(qb, qs, t):
            if t == 0:
                o_cur[qs] = ps_o.tile([P, 257], F32, name=f"ops_{qb}_{qs}", tag="o")
            nc.tensor.matmul(
                o_cur[qs],
                lhsT=e4_tiles[qb][:, 2 * t:2 * t + 2, qs * P:(qs + 1) * P],
                rhs=vv8[:, 2 * t:2 * t + 2, :],
                start=(t == 0), stop=(t == NPR - 1), perf_mode=DR,
            )

        def emit_ep_a(qb, qs):
            o = o_cur[qs]
            recip = sml.tile([P, 1], F32, name=f"rc_{qb}_{qs}", tag="recip")
            nc.vector.reciprocal(out=recip, in_=o[:, 256:257])
            attn = anp.tile([P, C], BF16, name=f"attn_{qb}_{qs}", tag="attn")
            nc.vector.tensor_scalar_mul(out=attn, in0=o[:, 0:256], scalar1=recip)
            return attn

        def emit_ep_b(qb, qs, attn):
            if qs == 0:
                tps_cur[qb] = ps_t.tile(
                    [P, CT, QB], BF16, name=f"tps_{qb}", tag="t"
                )
            tps = tps_cur[qb]
            for ct in range(CT):
                nc.tensor.transpose(
                    tps[:, ct, qs * P:(qs + 1) * P],
                    attn[:, ct * P:(ct + 1) * P],
                    eyeb,
                )
            if qs == 3:
                emit_qb_out(qb, tps_cur.pop(qb))

        def emit_qb_out(qb, tps):
            outt = outp.tile([P, CT, QB], F32, name=f"outt_{qb}", tag="outt")
            qs_ = slice(qb * QB, (qb + 1) * QB)
            for ct in range(CT):
                nc.vector.tensor_scalar(
                    out=outt[:, ct, :], in0=tps[:, ct, :],
                    scalar1=1.0 / 16.0, scalar2=bo_sb[:, ct:ct + 1],
                    op0=ALU.mult, op1=ALU.add,
                )
                nc.vector.tensor_add(
                    out=outt[:, ct, :], in0=outt[:, ct, :],
                    in1=x_slice(ct, qb * QB, QB),
                )
            out_r = out_d.rearrange("(t p) n -> p t n", p=P)
            nc.gpsimd.dma_start(out=out_r[:, :, qs_], in_=outt)

        # aux work interleaved into the S phase of each q-block:
        #   qb 0: the 32 VV matmul groups; qb >= 1: the 64 PV matmuls of qb-1.
        tps_cur = {}
        pending_b = []

        def aux_pv(qb_prev, i):  # i in 0..15 -> 4 PV matmuls per step
            new_b = []
            for k in range(4):
                idx = 4 * i + k
                qs, t = divmod(idx, NPR)
                emit_pv(qb_prev, qs, t)
                if t == NPR - 1:
                    attn = emit_ep_a(qb_prev, qs)
                    new_b.append((qb_prev, qs, attn))
            while pending_b:
                emit_ep_b(*pending_b.pop(0))
            pending_b.extend(new_b)

        for qb in range(NQB):
            e4_tiles[qb] = e4p.tile([P, NKT, QB], F8, name=f"e4_{qb}", tag="e4")
            if qb >= 3:
                del e4_tiles[qb - 3]
            for t in range(NPR):
                emit_s_pair(qb, t)
                if qb == 0:
                    emit_vv_pair(t)
                else:
                    aux_pv(qb - 1, t)
        for i in range(NPR):
            aux_pv(NQB - 1, i)
        while pending_b:
            emit_ep_b(*pending_b.pop(0))

    nc.compile()
    return nc


_NC = None


def _get_nc():
    global _NC
    if _NC is None:
        _NC = build_nc()
    return _NC


def _host_prep(x, w_q, b_q, w_k, b_k, w_v, b_v, w_o, b_o):
    x = np.ascontiguousarray(np.asarray(x, np.float32))
    B = x.shape[0]
    wq = np.asarray(w_q, np.float32)
    wk = np.asarray(w_k, np.float32)
    wv = np.asarray(w_v, np.float32)
    wo = np.asarray(w_o, np.float32)
    bq = np.asarray(b_q, np.float32)
    bk = np.asarray(b_k, np.float32)
    bv = np.asarray(b_v, np.float32)
    bo = np.asarray(b_o, np.float32)

    def to_pt(a):  # [C, ...] -> [P, CT, ...]
        return np.ascontiguousarray(
            a.reshape(CT, P, *a.shape[1:]).transpose(1, 0, *range(2, a.ndim + 1))
        )

    mt = (wk.T @ wq).astype(np.float32)       # lhsT[c, c'] = M[c', c]
    mt8 = to_pt((16.0 * mt).astype(F8NP))
    u = (wk.T @ bq).astype(np.float32)
    c0 = float(bq @ bk)
    w2 = (wo @ wv).astype(np.float32)
    b2 = (wo @ bv).astype(np.float32)
    w2t = np.zeros((C, 257), np.float32)
    w2t[:, :256] = 16.0 * w2.T
    w2t[:, 256] = 16.0 * u
    w2t8 = to_pt(w2t.astype(F8NP))
    bo = bo + b2   # sum_k softmax = 1 -> Wo b_v folds into the output bias

    xr = x.reshape(B, C, N)
    shared = {
        "mt8": mt8, "w2t8": w2t8, "bo": to_pt(bo),
    }
    in_maps = [{"x": np.ascontiguousarray(xr[i]), **shared} for i in range(B)]
    return x, in_maps


def kernel(x, w_q, b_q, w_k, b_k, w_v, b_v, w_o, b_o):
    x, in_maps = _host_prep(x, w_q, b_q, w_k, b_k, w_v, b_v, w_o, b_o)
    B = x.shape[0]
    nc = _get_nc()
    res = run_bass_kernel_spmd(nc, in_maps, core_ids=list(range(B)))
    global _LAST
    _LAST = res
    out = np.stack([res.results[i]["out"] for i in range(B)], axis=0)
    return out.reshape(x.shape).astype(np.float32)


_LAST = None


# revision 26
# speedup vs baseline: 1.1636x; 1.0049x over previous
"""AttentionBlock (GroupNorm + single-head full attention + residual) on 8 TRN2 cores.

Data-parallel: batch B=8, one sample per NeuronCore. Per core:
  x [256, 4096] f32 -> groupnorm -> h (fp8 e4m3)
  Algebraic folding (host-precomputed weight products):
    S[q,k] = q.k = sum_c h[c,q]*G2[c,k] + w[k] + c0
       G2 = M h + v,  M = Wq^T Wk, v = Wq^T b_k,  w[k] = (Wk^T b_q).h_k, c0 = b_q.b_k
    out_pre[q,co] = sum_k P[k,q]*VV[co,k],  VV = (Wo Wv) h + Wo b_v   (proj_out folded)
  All heavy matmuls run in fp8 e4m3 with MatmulPerfMode.DoubleRow (contraction
  over 2 k-subtiles per instruction, 2x PE throughput).  The per-k score bias
  w[k]+c0 is folded multiplicatively into VV (f[k] = exp(SCALE*(w[k]+c0)),
  sum_k e*f*vv == sum_k (e*f)*vv), which makes the softmax-exp bias a constant
  (-SHIFT) so each ACT exp instruction can span two PSUM banks (1024 wide).
  The softmax denominator rides as a ones-column of VV (scaled by f).  P^T
  layout [k, q] comes straight out of the S^T matmul so the 4096x4096 attention
  matrix is never transposed; only the final [4096, 256] attention output is
  transposed back to [c, n] via TensorE.
"""

import numpy as np
import ml_dtypes

import concourse.bacc as bacc
import concourse.bass as bass
import concourse.tile as tile
from concourse import mybir
from concourse.bass_utils import run_bass_kernel_spmd

F32 = mybir.dt.float32
BF16 = mybir.dt.bfloat16
F8 = mybir.dt.float8e4
AF = mybir.ActivationFunctionType
DR = mybir.MatmulPerfMode.DoubleRow
ALU = mybir.AluOpType
F8NP = ml_dtypes.float8_e4m3fn

C = 256          # channels
N = 4096         # spatial (64*64)
P = 128          # partitions
CT = C // P      # channel tiles (2)
NG = 8           # groups
GS = C // NG     # group size (32)
EPS = 1e-5
QB = 512         # queries per block
NQB = N // QB    # 8
NKT = N // P     # 32 k-tiles
NPR = NKT // 2   # 16 k-tile pairs
SCALE = 1.0 / np.sqrt(C)  # 1/16
SHIFT = 3.0      # global exp shift (softmax-invariant), keeps fp8 e in range


def _group_masks():
    g0 = np.zeros((P, NG), np.float32)
    g1 = np.zeros((P, NG), np.float32)
    for p in range(P):
        g0[p, p // GS] = 1.0
        g1[p, 4 + p // GS] = 1.0
    return g0, g1


def build_nc():
    nc = bacc.Bacc("TRN2", target_bir_lowering=False)

    x_d = nc.dram_tensor("x", [C, N], F32, kind="ExternalInput")
    wcat_d = nc.dram_tensor("wcat", [P, CT, 514], F8, kind="ExternalInput")
    fcat_d = nc.dram_tensor("fcat", [P, 146], F32, kind="ExternalInput")
    gcat_d = nc.dram_tensor("gcat", [NG, 2 * P], F32, kind="ExternalInput")
    out_d = nc.dram_tensor("out", [C, N], F32, kind="ExternalOutput")

    import contextlib
    with tile.TileContext(nc) as tc, contextlib.ExitStack() as ctx:
        cst = ctx.enter_context(tc.tile_pool(name="cst", bufs=1))
        big = ctx.enter_context(tc.tile_pool(name="big", bufs=1))
        e4p = ctx.enter_context(tc.tile_pool(name="e4p", bufs=3))
        anp = ctx.enter_context(tc.tile_pool(name="anp", bufs=4))
        outp = ctx.enter_context(tc.tile_pool(name="outp", bufs=2))
        sml = ctx.enter_context(tc.tile_pool(name="sml", bufs=2))
        ps_s = ctx.enter_context(tc.tile_pool(name="ps_s", bufs=2, space="PSUM"))
        ps_o = ctx.enter_context(tc.tile_pool(name="ps_o", bufs=3, space="PSUM"))
        ps_t = ctx.enter_context(tc.tile_pool(name="ps_t", bufs=1, space="PSUM"))

        # ---- const loads (3 packed DMAs) ----
        mt8_sb = cst.tile([P, CT, C], F8, name="mt8_sb")
        nc.sync.dma_start(out=mt8_sb, in_=wcat_d[:, :, 0:256])
        w2t8_sb = cst.tile([P, CT, 257], F8, name="w2t8_sb")
        nc.sync.dma_start(out=w2t8_sb, in_=wcat_d[:, :, 256:513])
        fcat_sb = cst.tile([P, 146], F32, name="fcat_sb")
        nc.sync.dma_start(out=fcat_sb, in_=fcat_d[:, :])
        bo_sb = fcat_sb[:, 0:2]
        g0_sb = fcat_sb[:, 2:10]
        g1_sb = fcat_sb[:, 10:18]
        eye_sb = fcat_sb[:, 18:146]
        gcat_sb = cst.tile([NG, 2 * P], F32, name="gcat_sb")
        nc.sync.dma_start(out=gcat_sb, in_=gcat_d[:, :])
        gt0_sb = gcat_sb[:, 0:P]
        gt1_sb = gcat_sb[:, P:2 * P]
        eyeb = cst.tile([P, P], BF16, name="eyeb")
        nc.vector.tensor_copy(out=eyeb, in_=eye_sb)

        eps_sb = cst.tile([NG, 1], F32, name="eps_sb")
        nc.vector.memset(eps_sb, EPS)
        warm = cst.tile([NG, 1], F32, name="warm")
        nc.scalar.activation(out=warm, in_=eps_sb, func=AF.Sqrt, bias=eps_sb)
        nshift = cst.tile([P, 1], F32, name="nshift")
        nc.vector.memset(nshift, -SHIFT)

        # ---- x load: 4 half-tiles (ct x half) in 1024-col chunks so the
        # groupnorm stats start as soon as each tile's DMAs land ----
        NH = N // 2
        x_t = {(ct, h): big.tile([P, NH], F32, name=f"x_{ct}_{h}")
               for ct in range(CT) for h in range(2)}
        x_r = x_d.rearrange("(t p) n -> p t n", p=P)
        for h in range(2):
            for xc in range(2):
                for ct in range(CT):
                    lo = xc * 1024
                    nc.sync.dma_start(
                        out=x_t[(ct, h)][:, lo:lo + 1024],
                        in_=x_r[:, ct, h * NH + lo:h * NH + lo + 1024],
                    )

        def x_slice(ct, n0, w):
            h, lo = divmod(n0, NH)
            return x_t[(ct, h)][:, lo:lo + w]

        # ---- groupnorm stats ----
        NSG = N // 512
        stats = sml.tile([P, CT, NSG, 6], F32, name="stats")
        mv = sml.tile([P, CT, 2], F32, name="mv")
        for h in range(2):
            for sg4 in range(NSG // 2):
                for ct in range(CT):
                    sg = h * (NSG // 2) + sg4
                    nc.vector.bn_stats(
                        out=stats[:, ct, sg, :],
                        in_=x_slice(ct, sg * 512, 512),
                    )
        for ct in range(CT):
            nc.vector.bn_aggr(out=mv[:, ct, :], in_=stats[:, ct, :, :])
        st3 = sml.tile([P, CT, 3], F32, name="st3")
        for ct in range(CT):
            nc.vector.tensor_copy(out=st3[:, ct, 0:2], in_=mv[:, ct, :])
            nc.vector.tensor_mul(
                out=st3[:, ct, 2:3], in0=mv[:, ct, 0:1], in1=mv[:, ct, 0:1]
            )
        gps = ps_s.tile([NG, 3], F32, name="gps", tag="s")
        nc.tensor.matmul(gps, lhsT=g0_sb, rhs=st3[:, 0, :], start=True, stop=False)
        nc.tensor.matmul(gps, lhsT=g1_sb, rhs=st3[:, 1, :], start=False, stop=True)
        gsb = sml.tile([NG, 3], F32, name="gsb")
        nc.vector.tensor_copy(out=gsb, in_=gps)
        gmean = sml.tile([NG, 1], F32, name="gmean")
        nc.vector.tensor_scalar_mul(out=gmean, in0=gsb[:, 0:1], scalar1=1.0 / GS)
        gtmp = sml.tile([NG, 1], F32, name="gtmp")
        nc.vector.tensor_add(out=gtmp, in0=gsb[:, 1:2], in1=gsb[:, 2:3])
        nc.vector.tensor_scalar_mul(out=gtmp, in0=gtmp, scalar1=1.0 / GS)
        gmsq = sml.tile([NG, 1], F32, name="gmsq")
        nc.vector.tensor_mul(out=gmsq, in0=gmean, in1=gmean)
        gvar = sml.tile([NG, 1], F32, name="gvar")
        nc.vector.tensor_sub(out=gvar, in0=gtmp, in1=gmsq)
        gstd = sml.tile([NG, 1], F32, name="gstd")
        nc.scalar.activation(out=gstd, in_=gvar, func=AF.Sqrt, bias=eps_sb)
        grstd = sml.tile([NG, 1], F32, name="grstd")
        nc.vector.reciprocal(out=grstd, in_=gstd)
        gpar = sml.tile([NG, 2], F32, name="gpar")
        nc.vector.tensor_copy(out=gpar[:, 0:1], in_=gmean)
        nc.vector.tensor_copy(out=gpar[:, 1:2], in_=grstd)
        mr_sb = sml.tile([P, CT, 2], F32, name="mr_sb")
        for ct, gt in ((0, gt0_sb), (1, gt1_sb)):
            bps = ps_s.tile([P, 2], F32, name=f"bps{ct}", tag="s")
            nc.tensor.matmul(bps, lhsT=gt, rhs=gpar, start=True, stop=True)
            nc.vector.tensor_copy(out=mr_sb[:, ct, :], in_=bps)
        # h = (x - mean) * rstd -> fp8, pipelined per 512-col block with G2
        hf8 = big.tile([P, CT, N], F8, name="hf8")
        g2f8 = big.tile([P, CT, N], F8, name="g2f8")
        for kb in range(NQB):
            ks = slice(kb * QB, (kb + 1) * QB)
            for ct in range(CT):
                nc.vector.tensor_scalar(
                    out=hf8[:, ct, ks],
                    in0=x_slice(ct, kb * QB, QB),
                    scalar1=mr_sb[:, ct, 0:1],
                    scalar2=mr_sb[:, ct, 1:2],
                    op0=ALU.subtract,
                    op1=ALU.mult,
                )
            g2ps = ps_s.tile([P, CT, QB], F32, name=f"g2ps_{kb}", tag="s")
            for ct in range(CT):
                nc.tensor.matmul(
                    g2ps[:, ct, :], lhsT=mt8_sb[:, :, ct * P:(ct + 1) * P],
                    rhs=hf8[:, :, ks], start=True, stop=True, perf_mode=DR,
                )
            nc.scalar.mul(g2f8[:, :, ks], g2ps, 1.0 / 16.0)

        # ---- VV projection + f = exp(SCALE*(w+c0)) folding ----
        # vv8[k, 0:256] = 16*VV[k, co]*f[k]; vv8[k, 256] = f[k]
        vv8 = big.tile([P, NKT, 257], F8, name="vv8")
        fz = big.tile([P, NKT], F32, name="fz")
        ftmp = big.tile([P, NKT], F32, name="ftmp")
        fex = big.tile([P, NKT], F32, name="fex")

        def emit_vv_mm(kt):
            vps = ps_o.tile([P, 257], F32, name=f"vps_{kt}", tag="o")
            ks = slice(kt * P, (kt + 1) * P)
            nc.tensor.matmul(vps, lhsT=hf8[:, :, ks], rhs=w2t8_sb,
                             start=True, stop=True, perf_mode=DR)
            return vps

        def emit_vv_pair(i):
            kt0, kt1 = 2 * i, 2 * i + 1
            vps0 = emit_vv_mm(kt0)
            vps1 = emit_vv_mm(kt1)
            for kt, vps in ((kt0, vps0), (kt1, vps1)):
                nc.vector.tensor_scalar_mul(
                    out=fz[:, kt:kt + 1], in0=vps[:, 256:257],
                    scalar1=float(SCALE / 16.0),
                )
            pr = slice(kt0, kt1 + 1)
            nc.vector.tensor_scalar(
                out=ftmp[:, pr], in0=fz[:, pr], scalar1=0.5, scalar2=1.0,
                op0=ALU.mult, op1=ALU.add,
            )
            nc.vector.tensor_mul(out=fex[:, pr], in0=ftmp[:, pr], in1=fz[:, pr])
            nc.vector.tensor_scalar_add(out=fex[:, pr], in0=fex[:, pr], scalar1=1.0)
            for kt, vps in ((kt0, vps0), (kt1, vps1)):
                nc.vector.tensor_scalar_mul(
                    out=vv8[:, kt, 0:256], in0=vps[:, 0:256],
                    scalar1=fex[:, kt:kt + 1],
                )
                nc.vector.tensor_scalar_mul(
                    out=vv8[:, kt, 256:257], in0=fex[:, kt:kt + 1], scalar1=1.0
                )

        # ---- attention ----
        e4_tiles = {}
        o_cur = {}

        def emit_s_pair(qb, t):
            qs_ = slice(qb * QB, (qb + 1) * QB)
            sp = ps_s.tile([P, 2, QB], F32, name=f"sps_{qb}_{t}", tag="s")
            for j in (0, 1):
                kt = 2 * t + j
                nc.tensor.matmul(
                    sp[:, j, :], lhsT=g2f8[:, :, kt * P:(kt + 1) * P],
                    rhs=hf8[:, :, qs_], start=True, stop=True, perf_mode=DR,
                )
            nc.scalar.activation(
                out=e4_tiles[qb][:, 2 * t:2 * t + 2, :], in_=sp,
                func=AF.Exp, scale=float(SCALE), bias=nshift,
            )

        def emit_pv(qb, qs, t):
            if t == 0:
                o_cur[qs] = ps_o.tile([P, 257], F32, name=f"ops_{qb}_{qs}", tag="o")
            nc.tensor.matmul(
                o_cur[qs],
                lhsT=e4_tiles[qb][:, 2 * t:2 * t + 2, qs * P:(qs + 1) * P],
                rhs=vv8[:, 2 * t:2 * t + 2, :],
                start=(t == 0), stop=(t == NPR - 1), perf_mode=DR,
            )

        def emit_ep_a(qb, qs):
            o = o_cur[qs]
            recip = sml.tile([P, 1], F32, name=f"rc_{qb}_{qs}", tag="recip")
            nc.vector.reciprocal(out=recip, in_=o[:, 256:257])
            attn = anp.tile([P, C], BF16, name=f"attn_{qb}_{qs}", tag="attn")
            nc.vector.tensor_scalar_mul(out=attn, in0=o[:, 0:256], scalar1=recip)
            return attn

        def emit_ep_b(qb, qs, attn):
            if qs == 0:
                tps_cur[qb] = ps_t.tile(
                    [P, CT, QB], BF16, name=f"tps_{qb}", tag="t"
                )
            tps = tps_cur[qb]
            for ct in range(CT):
                nc.tensor.transpose(
                    tps[:, ct, qs * P:(qs + 1) * P],
                    attn[:, ct * P:(ct + 1) * P],
                    eyeb,
                )
            if qs == 3:
                emit_qb_out(qb, tps_cur.pop(qb))

        def emit_qb_out(qb, tps):
            outt = outp.tile([P, CT, QB], F32, name=f"outt_{qb}", tag="outt")
            qs_ = slice(qb * QB, (qb + 1) * QB)
            for ct in range(CT):
                nc.vector.tensor_scalar(
                    out=outt[:, ct, :], in0=tps[:, ct, :],
                    scalar1=1.0 / 16.0, scalar2=bo_sb[:, ct:ct + 1],
                    op0=ALU.mult, op1=ALU.add,
                )
                nc.vector.tensor_add(
                    out=outt[:, ct, :], in0=outt[:, ct, :],
                    in1=x_slice(ct, qb * QB, QB),
                )
            out_r = out_d.rearrange("(t p) n -> p t n", p=P)
            nc.gpsimd.dma_start(out=out_r[:, :, qs_], in_=outt)

        # aux work interleaved into the S phase of each q-block:
        #   qb 0: the 32 VV matmul groups; qb >= 1: the 64 PV matmuls of qb-1.
        tps_cur = {}
        pending_b = []

        def aux_pv(qb_prev, i):  # i in 0..15 -> 4 PV matmuls per step
            new_b = []
            for k in range(4):
                idx = 4 * i + k
                qs, t = divmod(idx, NPR)
                emit_pv(qb_prev, qs, t)
                if t == NPR - 1:
                    attn = emit_ep_a(qb_prev, qs)
                    new_b.append((qb_prev, qs, attn))
            while pending_b:
                emit_ep_b(*pending_b.pop(0))
            pending_b.extend(new_b)

        for qb in range(NQB):
            e4_tiles[qb] = e4p.tile([P, NKT, QB], F8, name=f"e4_{qb}", tag="e4")
            if qb >= 3:
                del e4_tiles[qb - 3]
            for t in range(NPR):
                emit_s_pair(qb, t)
                if qb == 0:
                    emit_vv_pair(t)
                else:
                    aux_pv(qb - 1, t)
        for i in range(NPR):
            aux_pv(NQB - 1, i)
        while pending_b:
            emit_ep_b(*pending_b.pop(0))

    nc.compile()
    return nc


_NC = None


def _get_nc():
    global _NC
    if _NC is None:
        _NC = build_nc()
    return _NC


def _host_prep(x, w_q, b_q, w_k, b_k, w_v, b_v, w_o, b_o):
    x = np.ascontiguousarray(np.asarray(x, np.float32))
    B = x.shape[0]
    wq = np.asarray(w_q, np.float32)
    wk = np.asarray(w_k, np.float32)
    wv = np.asarray(w_v, np.float32)
    wo = np.asarray(w_o, np.float32)
    bq = np.asarray(b_q, np.float32)
    bk = np.asarray(b_k, np.float32)
    bv = np.asarray(b_v, np.float32)
    bo = np.asarray(b_o, np.float32)

    def to_pt(a):  # [C, ...] -> [P, CT, ...]
        return np.ascontiguousarray(
            a.reshape(CT, P, *a.shape[1:]).transpose(1, 0, *range(2, a.ndim + 1))
        )

    mt = (wk.T @ wq).astype(np.float32)       # lhsT[c, c'] = M[c', c]
    mt8 = to_pt((16.0 * mt).astype(F8NP))
    u = (wk.T @ bq).astype(np.float32)
    c0 = float(bq @ bk)
    w2 = (wo @ wv).astype(np.float32)
    b2 = (wo @ bv).astype(np.float32)
    w2t = np.zeros((C, 257), np.float32)
    w2t[:, :256] = 16.0 * w2.T
    w2t[:, 256] = 16.0 * u
    w2t8 = to_pt(w2t.astype(F8NP))
    bo = bo + b2   # sum_k softmax = 1 -> Wo b_v folds into the output bias
    pad = np.zeros((P, CT, 1), F8NP)
    wcat = np.concatenate([mt8, w2t8, pad], axis=2)     # [P, CT, 514] f8 (even stride)
    g0_np, g1_np = _group_masks()
    fcat = np.zeros((P, 146), np.float32)
    fcat[:, 0:2] = to_pt(bo)
    fcat[:, 2:10] = g0_np
    fcat[:, 10:18] = g1_np
    fcat[:, 18:146] = np.eye(P, dtype=np.float32)
    gcat = np.concatenate(
        [np.ascontiguousarray(g0_np.T), np.ascontiguousarray(g1_np.T)], axis=1
    )

    xr = x.reshape(B, C, N)
    shared = {
        "wcat": np.ascontiguousarray(wcat),
        "fcat": fcat,
        "gcat": np.ascontiguousarray(gcat.astype(np.float32)),
    }
    in_maps = [{"x": np.ascontiguousarray(xr[i]), **shared} for i in range(B)]
    return x, in_maps


def kernel(x, w_q, b_q, w_k, b_k, w_v, b_v, w_o, b_o):
    x, in_maps = _host_prep(x, w_q, b_q, w_k, b_k, w_v, b_v, w_o, b_o)
    B = x.shape[0]
    nc = _get_nc()
    res = run_bass_kernel_spmd(nc, in_maps, core_ids=list(range(B)))
    global _LAST
    _LAST = res
    out = np.stack([res.results[i]["out"] for i in range(B)], axis=0)
    return out.reshape(x.shape).astype(np.float32)


_LAST = None
